# revision 41
# baseline (speedup 1.0000x reference)
"""Trainium2 Bass kernel for RTMDet-style NMS detection head.

Full inputs: cls{0,1,2} [16,80,H,W], box{0,1,2} [16,4,H,W] (H,W = 80/40/20).
Output: (dets [16,100,5] f32, labels [16,100] int32) matching reference.

Sharding: pure data-parallel over batch; 2 images per NeuronCore x 8 cores.

Per-image device pipeline:
  1. DMA cls levels into a [112, 8448] SBUF tile (rows 0-79 = classes;
     rows 96-111 = host-packed box preds / priors px,py,stride / anchor-idx).
  2. PE-transpose [80,128] chunks -> PSUM, DVE reduce_max over classes
     -> per-anchor max logit in [128, 66] (anchor a = f*128 + p).
  3. DVE max/max_index top-8 per partition row; gpsimd kth_largest gives
     tau = 129th largest => mask of the top-128 anchors (the global
     top-129 always lie within their row's top-8 for this head).
  4. gpsimd sparse_gather compacts candidate anchor ids; ap_gather pulls
     the field rows for the 128 candidates; PE-transpose back.
  5. Decode boxes, sigmoid score (ACT), argmax label (max/max_index).
  6. Pairwise before/same-label/IoU matrices [128,128] on DVE
     (sign-test IoU, no division); greedy-NMS keep via matmul fixpoint.
  7. rank = matmul(keep, before); one-hot rank -> matmul gathers the
     top-100 rows in final sorted order; DMA out.
"""
import numpy as np

import concourse.bass as bass
import concourse.bacc as bacc
import concourse.mybir as mybir
from concourse import bass_isa
from concourse.tile import TileContext, add_dep_helper
from concourse.bass_utils import run_bass_kernel_spmd

F32 = mybir.dt.float32
I32 = mybir.dt.int32
I16 = mybir.dt.int16
U32 = mybir.dt.uint32
ALU = mybir.AluOpType
AXL = mybir.AxisListType
ACTF = mybir.ActivationFunctionType

N_CORES = 8
B_FULL = 16
B_CORE = B_FULL // N_CORES  # 2 images per core
NUM_CLASSES = 80
HW = ((80, 80), (40, 40), (20, 20))
STRIDES = (8, 16, 32)
LEVEL_SIZES = tuple(h * w for h, w in HW)      # 6400, 1600, 400
A = sum(LEVEL_SIZES)                           # 8400
APAD = 8448                                    # 66 * 128
NF = APAD // 128                               # 66
CAP = 128                                      # candidate capacity
NKEEP = 100
IOU_THR = 0.65
VALID_LOGIT = float(np.log(0.25 / 0.75))       # sigmoid(x) > 0.25
NEG = -1.0e30
FIX_ITERS = 2

# image tile rows
R_CLS = 0        # 0..79    class logits
R_AUX = 96       # 96..111  aux block (DMA'd from host-packed auxin)
ROWS = 112
# aux block rows (relative to R_AUX)
X_BOX = 0        # 0..3   box preds d0..d3
X_PP = 4         # 4..7   px, py, px, py
X_SS = 8         # 8..11  -st, -st, st, st
X_AIDX = 12
AUX_ROWS = 16    # 13..15 zero padding

# misc const layout [128, MISC_COLS]
C_EYE = 0        # cols 0..127   identity
C_P = 128        # col 128       partition index (f32)
C_IOTA = 130     # cols 130..257 every row holds 0..127
C_ONES = 258     # col 258       ones column
C_TILE = 260     # cols 260..387 rows 0..15: tiled identity L[k,q] = (q%16==k)
C_G = 388        # cols 388..395: G[p,f] = (f == p//16) column-select mask
C_ONER = 396     # cols 396..523: row 0 all-ones (PE row-broadcast lhsT)
C_E9 = 524       # cols 524..1419: 7 blocks [9,128]; block c has row SEL9[c] ones
MISC_COLS = 1420
# rows9 layout after transposing data[:, 4:13]:
#   0 sig, 1 labf, 2 a65, 3 aidx, 4 valid, 5 ox1, 6 oy1, 7 ox2, 8 oy2
# bc blocks:      0 ox1, 1 oy1, 2 ox2, 3 oy2, 4 sig, 5 a65, 6 aidx
SEL9 = (5, 6, 7, 8, 0, 2, 3)


def _host_consts():
    px = np.zeros(APAD, np.float32)
    py = np.zeros(APAD, np.float32)
    st = np.zeros(APAD, np.float32)
    off = 0
    for (h, w), s in zip(HW, STRIDES):
        ys, xs = np.meshgrid(np.arange(h, dtype=np.float32) * s,
                             np.arange(w, dtype=np.float32) * s, indexing='ij')
        n = h * w
        px[off:off + n] = xs.reshape(-1)
        py[off:off + n] = ys.reshape(-1)
        st[off:off + n] = float(s)
        off += n
    aidx = np.arange(APAD, dtype=np.float32)

    misc = np.zeros((128, MISC_COLS), np.float32)
    misc[:, C_EYE:C_EYE + 128] = np.eye(128, dtype=np.float32)
    misc[:, C_P] = np.arange(128, dtype=np.float32) + 1.0  # p+1 (see sel)
    misc[:, C_IOTA:C_IOTA + 128] = np.arange(128, dtype=np.float32)[None, :]
    misc[:, C_ONES] = 1.0
    q = np.arange(128)
    for k in range(16):
        misc[k, C_TILE:C_TILE + 128] = (q % 16 == k).astype(np.float32)
    misc[:, C_G:C_G + 8] = (np.arange(8)[None, :] ==
                            (np.arange(128) // 16)[:, None]).astype(np.float32)
    misc[0, C_ONER:C_ONER + 128] = 1.0
    for c, r in enumerate(SEL9):
        misc[r, C_E9 + 128 * c:C_E9 + 128 * (c + 1)] = 1.0
    return (px, py, st, aidx), misc


def pack_aux(box0, box1, box2):
    """[Bc,4,H,W] x3 -> anchor-major [Bc,APAD,16] with priors appended."""
    (px, py, st, aidx), _ = _host_consts()
    bc = box0.shape[0]
    aux = np.zeros((bc, APAD, AUX_ROWS), np.float32)
    off = 0
    for arr, n in zip((box0, box1, box2), LEVEL_SIZES):
        aux[:, off:off + n, 0:4] = arr.reshape(bc, 4, n).transpose(0, 2, 1)
        off += n
    aux[:, :, X_PP + 0] = px
    aux[:, :, X_PP + 1] = py
    aux[:, :, X_PP + 2] = px
    aux[:, :, X_PP + 3] = py
    aux[:, :, X_SS + 0] = -st
    aux[:, :, X_SS + 1] = -st
    aux[:, :, X_SS + 2] = st
    aux[:, :, X_SS + 3] = st
    aux[:, :, X_AIDX] = aidx
    return aux.reshape(bc * APAD, AUX_ROWS)


def _emit_frontA(nc, tc, pools, misc, dram, b):
    img_pool, work_pool, psum_pool, psum1_pool, psum2_pool = pools
    v = nc.vector
    g = nc.gpsimd

    img = img_pool.tile([80, APAD], F32, tag="img")

    # ---- loads ----
    qs = (nc.sync, nc.scalar)
    off = 0
    for li, n in enumerate(LEVEL_SIZES):
        src_ = dram[f'cls{li}'][b].rearrange("c h w -> c (h w)")
        h = n // len(qs)
        for qi, q in enumerate(qs):
            lo = qi * h
            hi = n if qi == len(qs) - 1 else (qi + 1) * h
            q.dma_start(img[R_CLS:R_CLS + 80, off + lo:off + hi], src_[:, lo:hi])
        off += n
    v.memset(img[R_CLS:R_CLS + 80, A:APAD], NEG)

    # ---- stage A: per-anchor class max -> maxlog [128, 66] ----
    # image 0: Pool (otherwise idle) takes the first POOL_T tiles via
    # partition_all_reduce on the native layout; DVE takes the rest via
    # PE-transpose + free-dim reduce.
    maxlog = work_pool.tile([128, NF], F32, tag="maxlog")
    first_reduce = None
    pool_t = 22
    if pool_t:
        # scrap rows are strided pool_t+1 so the AP stays 3-D (un-mergeable)
        scrap = work_pool.tile([80, 128 * (pool_t + 1)], F32, tag="parscrap")
        g.partition_all_reduce(
            scrap[0:80, :].rearrange("c (p f) -> c p f", f=pool_t + 1)[:, :, 0:pool_t],
            img[0:80, 0:128 * pool_t].rearrange("c (f p) -> c p f", p=128),
            channels=80, reduce_op=bass_isa.ReduceOp.max)
        nc.sync.dma_start(
            maxlog[:, 0:pool_t],
            scrap[0:1, :].rearrange("o (p f) -> o p f", f=pool_t + 1)[:, :, 0:pool_t])
    t0 = pool_t
    while t0 < NF:
        nt = min(12, NF - t0)
        bank = psum_pool.tile([128, 1024], F32, tag="tbank")
        for k in range(nt):
            t = t0 + k
            col = 80 * k + (32 if k >= 6 else 0)  # don't cross the bank seam
            nc.tensor.transpose(bank[:, col:col + 80],
                                img[R_CLS:R_CLS + 80, 128 * t:128 * t + 128],
                                misc[0:80, C_EYE:C_EYE + 80])
        n1 = min(nt, 6)
        ri = v.reduce_max(maxlog[:, t0:t0 + n1],
                          bank[:, 0:80 * n1].rearrange("p (g c) -> p g c", g=n1),
                          axis=AXL.X)
        if first_reduce is None:
            first_reduce = ri
        if nt > 6:
            n2 = nt - 6
            v.reduce_max(maxlog[:, t0 + 6:t0 + nt],
                         bank[:, 512:512 + 80 * n2].rearrange(
                             "p (g c) -> p g c", g=n2),
                         axis=AXL.X)
        t0 += nt

    # ---- top-8 per row + tau = 129th largest ----
    vals8 = work_pool.tile([128, 8], F32, tag="vals8")
    idx8u = work_pool.tile([128, 8], U32, tag="idx8u")
    v.max(out=vals8[:], in_=maxlog[:])
    v.max_index(out=idx8u[:], in_max=vals8[:], in_values=maxlog[:])

    # anchor ids + 1 = idx*128 + (p+1)
    anch = work_pool.tile([128, 8], F32, tag="anch")
    v.tensor_copy(anch[:], idx8u[:])  # u32 -> f32
    v.tensor_scalar(anch[:], anch[:], 128.0, misc[:, C_P:C_P + 1],
                    ALU.mult, ALU.add)
    sg_in = work_pool.tile([16, 72], F32, tag="sg_in")
    v.memset(sg_in[:, 64:72], float(A))           # sentinels -> pad anchor
    return dict(maxlog=maxlog, vals8=vals8, idx8u=idx8u, anch=anch,
                sg_in=sg_in, img=img, first_reduce=first_reduce)


def _emit_frontB(nc, tc, pools, misc, dram, b, st):
    img_pool, work_pool, psum_pool, psum1_pool, psum2_pool = pools
    v = nc.vector
    g = nc.gpsimd
    vals8, anch, sg_in = st['vals8'], st['anch'], st['sg_in']

    tau = work_pool.tile([1, 2], F32, tag="tau")
    g.kth_largest(tau[:], vals8[:], n_per_lane=8, k=CAP,
                  quantile=1.0 - 127.5 / 1023.0)
    taub = work_pool.tile([128, 1], F32, tag="taub")
    g.partition_broadcast(taub[:], tau[0:1, 1:2])

    # sel = mask*(a+1) - 1 (>=0 iff kept) — on Pool, inline with its chain
    mask8 = work_pool.tile([128, 8], F32, tag="mask8")
    sel = work_pool.tile([128, 8], F32, tag="sel")
    v.tensor_single_scalar(mask8[:], vals8[:], taub[:], ALU.is_gt)
    v.scalar_tensor_tensor(sel[:], mask8[:], 1.0, anch[:], ALU.mult, ALU.mult)
    st['sel_sub'] = v.tensor_single_scalar(sel[:], sel[:], 1.0, ALU.subtract)

    # ---- compaction ----
    sg_out = work_pool.tile([16, 16], F32, tag="sg_out")
    nfound = work_pool.tile([1, 1], U32, tag="nfound")
    nc.sync.dma_start(sg_in[0:16, 0:64], sel[:])  # [128,8] -> [16,64] stream
    st['sparse_inst'] = g.sparse_gather(sg_out[:], sg_in[:], num_found=nfound[:])

    # replicate the 128-idx wrap to all rows: tiled-identity matmul
    idxallp = psum1_pool.tile([128, 8], F32, tag="psA")
    nc.tensor.matmul(idxallp[:], misc[0:16, C_TILE:C_TILE + 128], sg_out[:, 0:8])
    idx80 = work_pool.tile([80, 8], I16, tag="idx80")
    v.tensor_copy(idx80[:], idxallp[0:80, :])
    # candidate j's anchor id on partition j: select column p//16, cast to i32
    cidxf = work_pool.tile([128, 8], F32, tag="cidxf")
    v.tensor_tensor(cidxf[:], idxallp[:], misc[:, C_G:C_G + 8], ALU.mult)
    cidx = work_pool.tile([128, 1], I32, tag="cidx")
    cidxs = work_pool.tile([128, 1], F32, tag="cidxs")
    v.reduce_sum(cidxs[:], cidxf[:], axis=AXL.X)
    if b:
        v.tensor_single_scalar(cidxs[:], cidxs[:], float(b * APAD), ALU.add)
    st['idx_cast'] = v.tensor_copy(cidx[:], cidxs[:])
    st.update(sel=sel, sg_out=sg_out, idx80=idx80, cidx=cidx, tau=tau)
    return st


def _emit_frontC(nc, tc, pools, misc, dram, b, st):
    img_pool, work_pool, psum_pool, psum1_pool, psum2_pool = pools
    v = nc.vector
    g = nc.gpsimd
    img, idx80, cidx = st['img'], st['idx80'], st['cidx']

    # ---- gather candidates: cls via ap_gather, aux via indirect DMA ----
    g80 = work_pool.tile([80, CAP], F32, tag="g80")
    g.ap_gather(g80[:], img[R_CLS:R_CLS + 80, 0:APAD], idx80[:],
                channels=80, num_elems=APAD, d=1, num_idxs=CAP)
    t80 = psum1_pool.tile([128, 80], F32, tag="psB")
    nc.tensor.transpose(t80[:], g80[:], misc[0:80, C_EYE:C_EYE + 80])
    cls128 = work_pool.tile([128, 80], F32, tag="cls128")
    nc.scalar.copy(cls128[:], t80[:])
    aux128 = work_pool.tile([128, AUX_ROWS], F32, tag="aux128")
    g.indirect_dma_start(
        out=aux128[:], out_offset=None,
        in_=dram['auxin'][:],
        in_offset=bass.IndirectOffsetOnAxis(ap=cidx[:, 0:1], axis=0))

    # ---- stage B: score/label/decode on the 128 candidates ----
    # data cols: 0 x1, 1 y1, 2 x2, 3 y2, 4 sig, 5 label, 6 .65*area(offset),
    #            7 aidx, 8 valid, 9..12 class-offset box coords
    data = work_pool.tile([128, 13], F32, tag="data")
    cv8 = work_pool.tile([128, 8], F32, tag="cv8")
    ci8 = work_pool.tile([128, 8], U32, tag="ci8")
    v.max(out=cv8[:], in_=cls128[:])
    v.max_index(out=ci8[:], in_max=cv8[:], in_values=cls128[:])
    v.tensor_copy(data[:, 5:6], ci8[:, 0:1])                     # label f32
    nc.scalar.activation(data[:, 4:5], cv8[:, 0:1], ACTF.Sigmoid)
    v.tensor_single_scalar(data[:, 8:9], cv8[:, 0:1], VALID_LOGIT, ALU.is_gt)

    # decode all 4 coords in two [128,4] ops: d*(+-s) + (px,py,px,py)
    v.tensor_tensor(data[:, 0:4], aux128[:, X_BOX:X_BOX + 4],
                    aux128[:, X_SS:X_SS + 4], ALU.mult)
    v.tensor_tensor(data[:, 0:4], data[:, 0:4],
                    aux128[:, X_PP:X_PP + 4], ALU.add)
    v.tensor_copy(data[:, 7:8], aux128[:, X_AIDX:X_AIDX + 1])
    # class-offset coords (batched-NMS trick, matches reference rounding)
    off = work_pool.tile([128, 1], F32, tag="off")
    v.tensor_single_scalar(off[:], data[:, 5:6], 8192.0, ALU.mult)
    v.tensor_scalar(data[:, 9:13], data[:, 0:4], off[:], None, ALU.add)
    # 0.65 * relu(ox2-ox1) * relu(oy2-oy1) (on offset coords, like reference)
    wq = work_pool.tile([128, 2], F32, tag="wq")
    v.scalar_tensor_tensor(wq[:], data[:, 9:11], -1.0, data[:, 11:13],
                           ALU.mult, ALU.add)
    v.tensor_single_scalar(wq[:], wq[:], 0.0, ALU.max)
    v.scalar_tensor_tensor(data[:, 6:7], wq[:, 0:1], IOU_THR, wq[:, 1:2],
                           ALU.mult, ALU.mult)                   # .65*w*h

    # ---- field rows: one 9-col transpose -> rows9 -> selector broadcasts ----
    r9p = psum1_pool.tile([9, 128], F32, tag="psA")
    nc.tensor.transpose(r9p[:], data[:, 4:13], misc[:, C_EYE:C_EYE + 128])
    rows9 = work_pool.tile([9, 128], F32, tag="rows9")
    nc.scalar.copy(rows9[:], r9p[:])
    bc = work_pool.tile([128, 7 * 128], F32, tag="bc")
    for h, w_ in ((0, 512), (1, 384)):
        bp = psum1_pool.tile([128, 512], F32, tag="psB")
        for j in range(w_ // 128):
            c = h * 4 + j
            nc.tensor.matmul(bp[:, 128 * j:128 * j + 128],
                             misc[0:9, C_E9 + 128 * c:C_E9 + 128 * (c + 1)],
                             rows9[:])
        if h == 0:
            nc.scalar.copy(bc[:, 512 * h:512 * h + w_], bp[:, 0:w_])
        else:
            v.tensor_copy(bc[:, 512 * h:512 * h + w_], bp[:, 0:w_])

    st.update(data=data, bc=bc, cls128=cls128, aux128=aux128)
    return st


def _emit_back(nc, tc, pools, misc, dram, b, st, debug=False):
    img_pool, work_pool, psum_pool, psum1_pool, psum2_pool = pools
    v = nc.vector
    g = nc.gpsimd
    data, bc = st['data'], st['bc']

    def rb(c):
        return bc[:, 128 * c:128 * c + 128]

    # ---- pairwise matrices [j(part), i(free)] ----
    Bef = work_pool.tile([128, 128], F32, tag="Bef")
    M = work_pool.tile([128, 128], F32, tag="M")
    w1 = work_pool.tile([128, 128], F32, tag="w1")
    w2 = work_pool.tile([128, 128], F32, tag="w2")
    w3 = work_pool.tile([128, 128], F32, tag="w3")

    v.tensor_single_scalar(Bef[:], rb(4), data[:, 4:5], ALU.is_lt)   # s_i < s_j
    v.tensor_single_scalar(w1[:], rb(4), data[:, 4:5], ALU.is_equal)
    v.tensor_single_scalar(w2[:], rb(6), data[:, 7:8], ALU.is_gt)    # idx_i > idx_j
    v.scalar_tensor_tensor(w1[:], w1[:], 1.0, w2[:], ALU.mult, ALU.mult)
    v.scalar_tensor_tensor(Bef[:], Bef[:], 1.0, w1[:], ALU.mult, ALU.add)

    # IoU sign test on offset coords: 1.65*inter - a65_i - a65_j > 0
    v.tensor_single_scalar(w1[:], rb(0), data[:, 9:10], ALU.max)      # xx1
    v.tensor_single_scalar(w2[:], rb(2), data[:, 11:12], ALU.min)     # xx2
    v.scalar_tensor_tensor(w1[:], w1[:], -1.0, w2[:], ALU.mult, ALU.add)  # w
    v.tensor_single_scalar(w1[:], w1[:], 0.0, ALU.max)
    v.tensor_single_scalar(w2[:], rb(1), data[:, 10:11], ALU.max)     # yy1
    v.tensor_single_scalar(w3[:], rb(3), data[:, 12:13], ALU.min)     # yy2
    v.scalar_tensor_tensor(w2[:], w2[:], -1.0, w3[:], ALU.mult, ALU.add)  # h
    v.tensor_single_scalar(w2[:], w2[:], 0.0, ALU.max)
    v.scalar_tensor_tensor(w1[:], w1[:], 1.0, w2[:], ALU.mult, ALU.mult)  # inter
    blend = float(1.0 + IOU_THR) / IOU_THR
    v.scalar_tensor_tensor(w1[:], w1[:], blend, rb(5), ALU.mult, ALU.subtract)
    v.tensor_scalar(w1[:], w1[:], data[:, 6:7], 0.0, ALU.subtract, ALU.is_gt)
    v.scalar_tensor_tensor(M[:], w1[:], 1.0, Bef[:], ALU.mult, ALU.mult)

    # ---- greedy NMS keep via matmul fixpoint (column form) ----
    # cnt_col = M^T @ k: out[i] = sum_j M[j,i] k[j]; k' = (cnt==0) * valid
    kcol = work_pool.tile([128, 1], F32, tag="kcol")
    v.tensor_copy(kcol[:], data[:, 8:9])
    for _ in range(FIX_ITERS):
        cntT = psum2_pool.tile([128, 1], F32, tag="psC")
        nc.tensor.matmul(cntT[:], M[:], kcol[:])
        v.tensor_scalar(kcol[:], cntT[:], 0.0, data[:, 8:9],
                        ALU.is_equal, ALU.mult)

    # ---- rank + one-hot output ----
    rankT = psum2_pool.tile([128, 1], F32, tag="psC")
    nc.tensor.matmul(rankT[:], Bef[:], kcol[:])
    OH = work_pool.tile([128, NKEEP], F32, tag="OH")
    v.tensor_scalar(OH[:], misc[:, C_IOTA:C_IOTA + NKEEP],
                    rankT[:], kcol[:], ALU.is_equal, ALU.mult)

    out6 = psum2_pool.tile([NKEEP, 6], F32, tag="psC")
    nc.tensor.matmul(out6[:], OH[:], data[:, 0:6])
    out6s = work_pool.tile([NKEEP, 6], F32, tag="out6s")
    nc.scalar.copy(out6s[:], out6[:])
    lab32 = work_pool.tile([NKEEP, 1], I32, tag="lab32")
    v.tensor_copy(lab32[:], out6s[:, 5:6])
    nc.sync.dma_start(dram['dets'][b], out6s[:, 0:5])
    nc.scalar.dma_start(dram['labels'][b], lab32[:])
    if debug and b == 0:
        for nm, t in (('d_maxlog', st['maxlog']), ('d_vals8', st['vals8']),
                      ('d_anch', st['anch']), ('d_tau', st['tau']),
                      ('d_sel', st['sel']), ('d_sgout', st['sg_out']),
                      ('d_cand', st['cls128']), ('d_aux', st['aux128']),
                      ('d_data', data), ('d_bc', bc),
                      ('d_kcol', kcol), ('d_M', M),
                      ('d_Bef', Bef)):
            nc.sync.dma_start(dram[nm][:], t[:])


DEBUG_SHAPES = {
    'd_maxlog': [128, NF], 'd_vals8': [128, 8], 'd_anch': [128, 8],
    'd_tau': [1, 2], 'd_sel': [128, 8], 'd_sgout': [16, 16],
    'd_cand': [128, 80], 'd_aux': [128, 16], 'd_data': [128, 9],
    'd_bc': [128, 7 * 128], 'd_kcol': [128, 1],
    'd_M': [128, 128], 'd_Bef': [128, 128],
}


def build_nc(debug=False):
    nc = bacc.Bacc()
    dram = {}
    for li, (h, w) in enumerate(HW):
        dram[f'cls{li}'] = nc.dram_tensor(f'cls{li}', [B_CORE, NUM_CLASSES, h, w],
                                          F32, kind="ExternalInput")
    dram['auxin'] = nc.dram_tensor('auxin', [B_CORE * APAD, AUX_ROWS], F32,
                                   kind="ExternalInput")
    misc_d = nc.dram_tensor('misc_const', [128, MISC_COLS], F32,
                            kind="ExternalInput")
    dram['dets'] = nc.dram_tensor('dets', [B_CORE, NKEEP, 5], F32,
                                  kind="ExternalOutput")
    dram['labels'] = nc.dram_tensor('labels', [B_CORE, NKEEP], I32,
                                    kind="ExternalOutput")
    if debug:
        for nm, shp in DEBUG_SHAPES.items():
            dram[nm] = nc.dram_tensor(nm, shp, F32, kind="ExternalOutput")

    with TileContext(nc) as tc:
        with (
            tc.tile_pool(name="img", bufs=2) as img_pool,
            tc.tile_pool(name="work", bufs=2) as work_pool,
            tc.tile_pool(name="const", bufs=1) as const_pool,
            tc.tile_pool(name="psum", bufs=2, space="PSUM") as psum_pool,
            tc.tile_pool(name="psum1", bufs=1, space="PSUM") as psum1_pool,
            tc.tile_pool(name="psum2", bufs=2, space="PSUM") as psum2_pool,
        ):
            misc = const_pool.tile([128, MISC_COLS], F32)
            nc.sync.dma_start(misc[:], misc_d[:])
            warm = const_pool.tile([128, 1], F32)
            nc.scalar.activation(warm[:], misc[:, C_P:C_P + 1], ACTF.Sigmoid)
            pools = (img_pool, work_pool, psum_pool, psum1_pool, psum2_pool)
            sts = []
            for b in range(B_CORE):
                st = _emit_frontA(nc, tc, pools, misc, dram, b)
                _emit_frontB(nc, tc, pools, misc, dram, b, st)
                sts.append(st)

            for b in range(B_CORE):
                _emit_frontC(nc, tc, pools, misc, dram, b, sts[b])
            for b in range(B_CORE):
                _emit_back(nc, tc, pools, misc, dram, b, sts[b], debug=debug)
    nc.compile()
    return nc


_NC = None


def _get_nc():
    global _NC
    if _NC is None:
        _NC = build_nc()
    return _NC


def _make_in_maps(inputs):
    _, misc = _host_consts()
    in_maps = []
    for c in range(N_CORES):
        s = slice(c * B_CORE, (c + 1) * B_CORE)
        m = {k: np.ascontiguousarray(np.asarray(inputs[k])[s], dtype=np.float32)
             for k in ('cls0', 'cls1', 'cls2')}
        m['auxin'] = pack_aux(np.asarray(inputs['box0'])[s],
                              np.asarray(inputs['box1'])[s],
                              np.asarray(inputs['box2'])[s])
        m['misc_const'] = misc
        in_maps.append(m)
    return in_maps


def kernel(**inputs):
    res = run_bass_kernel_spmd(_get_nc(), _make_in_maps(inputs),
                               list(range(N_CORES))).results
    dets = np.concatenate([r['dets'] for r in res], axis=0)
    labels = np.concatenate([r['labels'] for r in res], axis=0)
    return dets, labels


def run_traced(inputs, **kw):
    """Like kernel() but returns (dets, labels, BassKernelResults) with
    NTFF profiling enabled (for development/timing only)."""
    r = run_bass_kernel_spmd(_get_nc(), _make_in_maps(inputs),
                             list(range(N_CORES)), trace=True, **kw)
    dets = np.concatenate([x['dets'] for x in r.results], axis=0)
    labels = np.concatenate([x['labels'] for x in r.results], axis=0)
    return dets, labels, r
